# revision 2
# baseline (speedup 1.0000x reference)
"""Trainium2 Bass kernel for nn_CAKernel_47459388621075 (v2: pair-column layout).

10 steps of x = clip(x + 0.1*relu(conv5x5_circular(x, W)), 0, 1) on
x:(16,3,1024,1024) f32, W:(3,3,5,5) f32.

Sharding: batch-parallel over 8 NeuronCores (2 images/core), no collectives.

v2 layout: columns are compressed in pairs into channels. A row-record is
6 partitions (p = 3*j + ci, j in {0,1}) over 512 compressed columns, so the
5-tap dx contraction needs only THREE column-shifted matmuls (vs 5 in v1):
out col-pair c reads pairs c-1, c, c+1. dy is banded into K as before.

Blocks of B=17 rows (60 full + one 4-row tail): K = 6*(B+4) = 126, M = 6*B
= 102, N = 512. Per block per step: 3 fp16 matmuls (instead of v1's
10-matmul equivalent of 5 shifts x 2 groups at M=114) -> 1.49x fewer PE
cycles. Per-image state is ONE 3D SBUF tile q[126, 61, 514] (fp16, 0.1
folded into the weights; tile col 0/513 are circular column wraps, halo
rows of neighbors are duplicated across tiles).

Pointwise per block-pair: ACT relu (PSUM->fp16), DVE add (+state), DVE
min(.,1) written back into the state tile in place. Column wraps are fixed
by 4 chunked strided copies per image-step; row halos by 4+4 chunked
batched SBUF->SBUF DMAs plus 2 small wrap DMAs (circular).
"""
import sys

sys.path.insert(0, "/opt/trn_rl_repo")

import numpy as np

N_CORES = 8
H = 1024
W_ = 1024
C2 = 512  # compressed columns
B = 17
NBF = 60  # full blocks
BT = 4  # tail block rows
NB = NBF + 1
TS = 514  # tile free stride (1 + 512 + 1)
BLKS = [B] * NBF + [BT]
# pair-chunks: tiles [0,16), [16,32), [32,48), [48,61)
CHUNKS = [(0, 16), (16, 32), (32, 48), (48, NB)]


def make_stationary(W: np.ndarray, Bb: int) -> np.ndarray:
    """[126, 3, 6*Bb] f32: S[p_in, delta+1, m], 0.1 folded in.

    Partition layout (engine APs must start at partition 0): center rows
    v in [0,Bb) at p = 6v+ch; top halo rows v in {-2,-1} at 102+6(v+2)+ch;
    bottom halo rows v in {Bb, Bb+1} at 114+6(v-Bb)+ch.
    """
    assert W.shape == (3, 3, 5, 5)
    MP = 6 * Bb
    S = np.zeros((126, 3, MP), dtype=np.float32)
    for r in range(Bb):
        for dy in range(5):
            v = r + dy - 2
            if 0 <= v < Bb:
                pb = 6 * v
            elif v < 0:
                pb = 102 + 6 * (v + 2)
            else:
                pb = 114 + 6 * (v - Bb)
            for jo in range(2):
                for dx in range(5):
                    t = jo + dx - 2
                    ji = t % 2
                    d = (t - ji) // 2
                    for ci in range(3):
                        for co in range(3):
                            S[pb + 3 * ji + ci, d + 1, 6 * r + 3 * jo + co] = (
                                0.1 * W[co, ci, dy, dx]
                            )
    return S


def prep_x(x: np.ndarray) -> np.ndarray:
    """(n,3,H,W) f32 -> (n, 126, NB, TS) f16 block records."""
    n = x.shape[0]
    # P[n, row, 3*j+ci, c]
    P = (
        x.astype(np.float16)
        .reshape(n, 3, H, C2, 2)
        .transpose(0, 2, 4, 1, 3)
        .reshape(n, H, 6, C2)
    )
    out = np.zeros((n, 126, NB, TS), dtype=np.float16)
    for b in range(NB):
        Bb = BLKS[b]
        r0 = B * b
        center = [(r0 + i) % H for i in range(Bb)]
        halo = [(r0 - 2) % H, (r0 - 1) % H, (r0 + Bb) % H, (r0 + Bb + 1) % H]
        crec = P[:, center].reshape(n, 6 * Bb, C2)
        hrec = P[:, halo].reshape(n, 24, C2)
        for rec, p0 in ((crec, 0), (hrec, 102)):
            pp = rec.shape[1]
            out[:, p0 : p0 + pp, b, 1 : C2 + 1] = rec
            out[:, p0 : p0 + pp, b, 0] = rec[:, :, C2 - 1]
            out[:, p0 : p0 + pp, b, C2 + 1] = rec[:, :, 0]
    return out


def unprep_y(y: np.ndarray) -> np.ndarray:
    """(n, 31, 102, 2, 512) f16 -> (n, 3, H, W) f32."""
    n = y.shape[0]
    out = np.empty((n, 3, H, W_), dtype=np.float32)
    for b in range(NB):
        Bb = BLKS[b]
        rec = (
            y[:, b // 2, : 6 * Bb, b % 2, :]
            .astype(np.float32)
            .reshape(n, Bb, 2, 3, C2)
        )
        # [n, r, j, co, c] -> [n, co, r, c, j]
        out[:, :, B * b : B * b + Bb, :] = rec.transpose(0, 3, 1, 4, 2).reshape(
            n, 3, Bb, W_
        )
    return out


def build_body(tc, xq_ap, lw_ap, lwt_ap, y_ap, n_img, steps):
    from contextlib import ExitStack

    from concourse import mybir

    nc = tc.nc
    f32 = mybir.dt.float32
    f16 = mybir.dt.float16
    Relu = mybir.ActivationFunctionType.Relu

    ctx = ExitStack()
    const_pool = ctx.enter_context(tc.tile_pool(name="const", bufs=1))
    state_pool = ctx.enter_context(tc.tile_pool(name="state", bufs=1))
    t_pool = ctx.enter_context(tc.tile_pool(name="t", bufs=6))
    u_pool = ctx.enter_context(tc.tile_pool(name="u", bufs=6))
    y_pool = ctx.enter_context(tc.tile_pool(name="yst", bufs=6))
    psum_pool = ctx.enter_context(tc.tile_pool(name="psum", bufs=4, space="PSUM"))

    lw = const_pool.tile([126, 3, 102], f16)
    nc.scalar.dma_start(lw[:], lw_ap[:, :, :])
    lwt = const_pool.tile([126, 3, 24], f16)
    nc.scalar.dma_start(lwt[:], lwt_ap[:, :, :])

    q = [state_pool.tile([126, NB, TS], f16, name=f"q{img}") for img in range(n_img)]
    # initial load: small leading chunk for img0 so block 0's matmuls start
    # within ~2us, remainder staggered across 4 DMA queues in block order.
    load_chunks = [(0, 4), (4, 16), (16, 32), (32, 48), (48, NB)]
    for img in range(n_img):
        for t0, t1 in load_chunks:
            nc.sync.dma_start(q[img][0:126, t0:t1, :], xq_ap[img, 0:126, t0:t1, :])

    def do_pair(img, s, b, nb2):
        """Process blocks b..b+nb2-1 (nb2 in {1,2}); b+1 only if full."""
        last = s == steps - 1
        qi = q[img]
        ps = psum_pool.tile([102, 2, C2], f32, name="ps")
        for h in range(nb2):
            bb = b + h
            Bb = BLKS[bb]
            MP = 6 * Bb
            lww = lw if Bb == B else lwt
            for d in range(3):
                nc.tensor.matmul(
                    ps[0:MP, h, 0:C2],
                    lww[0:126, d, 0:MP],
                    qi[0:126, bb, d : d + C2],
                    start=(d == 0),
                    stop=(d == 2),
                )
        MPP = 102 if nb2 == 2 else 6 * BLKS[b]
        t = t_pool.tile([102, 2, C2], f16)
        nc.scalar.activation(t[0:MPP, 0:nb2, :], ps[0:MPP, 0:nb2, :], Relu)
        u = u_pool.tile([102, 2, C2], f16)
        nc.vector.tensor_add(
            u[0:MPP, 0:nb2, :],
            t[0:MPP, 0:nb2, :],
            qi[0:MPP, b : b + nb2, 1 : C2 + 1],
        )
        if last:
            yt = y_pool.tile([102, 2, C2], f16)
            nc.vector.tensor_scalar_min(yt[0:MPP, 0:nb2, :], u[0:MPP, 0:nb2, :], 1.0)
            nc.sync.dma_start(y_ap[img, b // 2, 0:MPP, 0:nb2, :], yt[0:MPP, 0:nb2, :])
        else:
            nc.vector.tensor_scalar_min(
                qi[0:MPP, b : b + nb2, 1 : C2 + 1], u[0:MPP, 0:nb2, :], 1.0
            )

    def colfix(img, t0, t1, ptop):
        # tile cols 0 <- 512, 513 <- 1 (circular column wrap), center rows
        qi = q[img]
        nc.gpsimd.tensor_copy(
            qi[0:ptop, t0:t1, 0 : TS : TS - 1], qi[0:ptop, t0:t1, C2 :: -(C2 - 1)]
        )

    def halo_dmas(img, k):
        # top halos: dst tiles 16k+1..16k+16  <- src tiles 16k..16k+15
        #   (dst partitions 102:114 = rows r0-2,r0-1; src = prev center
        #    rows 15,16 = partitions 90:102)
        # bottom halos: dst tiles 16k..16k+15 <- src tiles 16k+1..16k+16
        #   (dst partitions 114:126 = rows r0+B,r0+B+1; src = next center
        #    rows 0,1 = partitions 0:12)
        qi = q[img]
        # img1's halos ride the gpsimd SWDGE queue so the last step's
        # y-store burst on the sync queue can't delay them
        eng = nc.sync if img == 0 else nc.gpsimd
        t0 = 16 * k
        t1 = min(16 * (k + 1), NB - 1)
        eng.dma_start(qi[102:114, t0 + 1 : t1 + 1, :], qi[90:102, t0:t1, :])
        eng.dma_start(qi[114:126, t0:t1, :], qi[0:12, t0 + 1 : t1 + 1, :])

    for s in range(steps):
        last = s == steps - 1
        for img in range(n_img):
            for k, (t0, t1) in enumerate(CHUNKS):
                bb = t0
                while bb < t1:
                    nb2 = (
                        2
                        if bb + 1 < t1 and BLKS[bb] == B and BLKS[bb + 1] == B
                        else 1
                    )
                    do_pair(img, s, bb, nb2)
                    bb += nb2
                if not last:
                    if k < 3:
                        colfix(img, t0, t1, 102)
                    else:
                        colfix(img, t0, NB - 1, 102)
                        colfix(img, NB - 1, NB, 24)
                    if k >= 1:
                        halo_dmas(img, k - 1)
            if not last:
                halo_dmas(img, 3)
                qi = q[img]
                eng = nc.sync if img == 0 else nc.gpsimd
                # circular wraps: tile0 top halo (102:114) <- tail center
                # rows 1022/1023 (partitions 12:24); tail bottom halo
                # (114:126) <- tile0 center rows 0/1 (partitions 0:12)
                eng.dma_start(qi[102:114, 0, :], qi[12:24, NB - 1, :])
                eng.dma_start(qi[114:126, NB - 1, :], qi[0:12, 0, :])

    ctx.close()


_PROGRAM_CACHE = {}


def _build_program(n_img, steps):
    key = (n_img, steps)
    if key in _PROGRAM_CACHE:
        return _PROGRAM_CACHE[key]
    import concourse.tile as tile
    from concourse import bacc, mybir

    nc = bacc.Bacc(
        "TRN2",
        target_bir_lowering=False,
        debug=False,
        enable_asserts=False,
        num_devices=N_CORES,
    )
    f16 = mybir.dt.float16
    xq_ap = nc.dram_tensor("xq", (n_img, 126, NB, TS), f16, kind="ExternalInput").ap()
    lw_ap = nc.dram_tensor("lw", (126, 3, 102), f16, kind="ExternalInput").ap()
    lwt_ap = nc.dram_tensor("lwt", (126, 3, 24), f16, kind="ExternalInput").ap()
    y_ap = nc.dram_tensor(
        "y", (n_img, (NB + 1) // 2, 102, 2, C2), f16, kind="ExternalOutput"
    ).ap()
    with tile.TileContext(nc) as tc:
        build_body(tc, xq_ap, lw_ap, lwt_ap, y_ap, n_img, steps)
    nc.compile()
    _PROGRAM_CACHE[key] = nc
    return nc


def kernel(x: np.ndarray, W: np.ndarray, steps) -> np.ndarray:
    from concourse.bass_utils import run_bass_kernel_spmd

    x = np.ascontiguousarray(np.asarray(x), dtype=np.float32)
    W = np.asarray(W, dtype=np.float32)
    steps = int(steps)
    n, c, Hx, Wx = x.shape
    assert c == 3 and Hx == H and Wx == W_ and n % N_CORES == 0
    if steps == 0:
        return x
    per = n // N_CORES

    nc = _build_program(per, steps)
    xq = prep_x(x)
    lw = make_stationary(W, B).astype(np.float16)
    lwt = make_stationary(W, BT).astype(np.float16)
    in_maps = [
        {"xq": xq[i * per : (i + 1) * per], "lw": lw, "lwt": lwt}
        for i in range(N_CORES)
    ]
    res = run_bass_kernel_spmd(nc, in_maps, core_ids=list(range(N_CORES)))
    y = np.concatenate([res.results[i]["y"] for i in range(N_CORES)], axis=0)
    return unprep_y(y)
